# revision 1
# baseline (speedup 1.0000x reference)
"""BioLinearAttention (ELU+1 linear attention) on 8 TRN2 NeuronCores.

Sharding: token-parallel. The (B, T) = (4, 4096) grid flattens to 16384 rows;
each core owns 2048 contiguous rows (core c holds batch c//2's half). Each core
computes q/k/v projections for its rows, partial kv = k'^T v and k_sum over its
rows, then a pairwise AllReduce (cores 2b, 2b+1 share batch b) completes the
per-batch kv / k_sum. Stage 2 computes y = (q' kv) / (q'.k_sum) and the output
projection for the core's rows; host-side gather is a pure concat.

Layouts: host pre-transposes x and the weights so the contraction dim always
sits on SBUF partitions (no on-chip transposes). All matmuls run in float32r
(fp32 with 11-bit mantissa, full PE rate at N>=256). f32r PSUM accumulation
groups must stay contiguous on the PE (interleaving corrupts them), so the
cross-tile kv/k_sum accumulation happens in SBUF via DVE adds.
"""

import sys
import types

import numpy as np

B, T, C = 4, 4096, 1024
H, D = 16, 64
N_CORES = 8
ROWS = B * T
RPC = ROWS // N_CORES  # rows per core: 2048
NT = RPC // 128  # 128-token tiles per core: 16
NST = RPC // 512  # 512-token super-tiles per core: 4

_CACHE = {}


def _ensure_hook_shim():
    """bass_utils imports antenv.axon_hooks when BASS_TRACE is set; the image
    lacks that module. Provide a no-op shim unless one is already installed."""
    if "antenv.axon_hooks" in sys.modules:
        return
    try:
        import antenv
    except ImportError:
        return
    mod = types.ModuleType("antenv.axon_hooks")
    mod._hook = None
    mod.set_axon_ntff_profile_hook = lambda h: setattr(mod, "_hook", h)
    mod.get_axon_ntff_profile_hook = lambda: mod._hook
    sys.modules["antenv.axon_hooks"] = mod
    antenv.axon_hooks = mod


def _build(with_bias):
    key = ("nc", with_bias)
    if key in _CACHE:
        return _CACHE[key]

    import concourse.bacc as bacc
    import concourse.mybir as mybir
    from concourse.tile import TileContext

    F32 = mybir.dt.float32
    F32R = mybir.dt.float32r
    BF16 = mybir.dt.bfloat16

    nc = bacc.Bacc("TRN2", num_devices=N_CORES, debug=False)

    xt = nc.dram_tensor("xt", [C, RPC], F32R, kind="ExternalInput")
    wkvt = nc.dram_tensor("wkvt", [C, 2 * C], F32R, kind="ExternalInput")
    wqt = nc.dram_tensor("wqt", [C, C], F32R, kind="ExternalInput")
    wct = nc.dram_tensor("wct", [C, C], F32R, kind="ExternalInput")
    if with_bias:
        bkv = nc.dram_tensor("bkv", [1, 2 * C], F32R, kind="ExternalInput")
        bq = nc.dram_tensor("bq", [1, C], F32R, kind="ExternalInput")
        bc = nc.dram_tensor("bc", [1, C], F32R, kind="ExternalInput")
    out = nc.dram_tensor("out", [RPC, C], F32, kind="ExternalOutput")
    # kv | k_sum partials, SBUF layout: [d (row 64 = k_sum), (h, e)]
    cc_in = nc.dram_tensor("cc_in", [D + 1, C], F32, kind="Internal")
    cc_out = nc.dram_tensor("cc_out", [D + 1, C], F32, kind="Internal")
    groups = [[0, 1], [2, 3], [4, 5], [6, 7]]

    with TileContext(nc) as tc:
        with (
            tc.tile_pool(name="const", bufs=1) as cst,
            tc.tile_pool(name="wq", bufs=1) as wqp,
            tc.tile_pool(name="kvkr", bufs=1) as kvp,
            tc.tile_pool(name="xt2a", bufs=1) as xp2a,
            tc.tile_pool(name="krp", bufs=1, space="PSUM") as krpp,
        ):
            onecol = cst.tile([128, 1], F32R)
            ones_row = cst.tile([1, 64], F32R)
            if with_bias:
                ones = cst.tile([1, 512], F32R)
                bq_sb = cst.tile([1, C], F32R)
                nc.sync.dma_start(bq_sb[:], bq.ap())
                bc_sb = cst.tile([1, C], F32R)
                nc.sync.dma_start(bc_sb[:], bc.ap())

            wq_sb = wqp.tile([128, 8, C], F32R)

            # per-head stationary [krep_h | kv_h]: head h lives in partitions
            # (h%2)*64..+64 (matching q^T's head layout), free h*128..(h+1)*128.
            # krep first => the y/den matmul puts the denominator in rows 0:63
            # (reciprocal_approx_fast requires base-partition-0 input).
            kvkr = kvp.tile([128, H * 128], F32R)
            kvt = kvp.tile([D + 1, C], F32)
            ksum_r = kvp.tile([1, C], F32R)

            # ---------------- stage 1: k/v projections, kv & k_sum partials
            with (
                tc.tile_pool(name="wkv", bufs=1) as wkvp,
                tc.tile_pool(name="xt1", bufs=3) as xp1,
                tc.tile_pool(name="kv1", bufs=2) as kvp1,
                tc.tile_pool(name="tmp1", bufs=2) as tp1,
                tc.tile_pool(name="xfer", bufs=1) as xfr,
                tc.tile_pool(name="ps1", bufs=2, space="PSUM") as ps1,
                tc.tile_pool(name="kvps", bufs=2, space="PSUM") as kvpsp,
            ):
                pre_x = {}
                for tt0 in range(2):
                    px = xp1.tile([128, 8, 128], F32R)
                    nc.sync.dma_start(
                        px[:],
                        xt.ap().rearrange("(c p) t -> p c t", p=128)[
                            :, :, tt0 * 128 : (tt0 + 1) * 128
                        ],
                    )
                    pre_x[tt0] = px
                wkv_sb = wkvp.tile([128, 8, 2 * C], F32R)
                for ic in range(8):
                    nc.sync.dma_start(
                        wkv_sb[:, ic, :],
                        wkvt.ap().rearrange("(c p) n -> p c n", p=128)[:, ic, :],
                    )
                boot_f = xfr.tile([128, 512], F32)
                nc.vector.memset(boot_f[:], 1.0)
                nc.vector.tensor_copy(onecol[:], boot_f[:, 0:1])
                nc.vector.tensor_copy(ones_row[:], boot_f[0:1, 0:64])
                if with_bias:
                    nc.vector.tensor_copy(ones[:], boot_f[0:1, :])
                    bkv_sb = xfr.tile([1, 2 * C], F32R)
                    nc.sync.dma_start(bkv_sb[:], bkv.ap())

                # kv | k_sum accumulator (SBUF, DVE-accumulated across tiles)
                kvs = xfr.tile([D + 1, C], F32)

                prev = None
                for tt in range(NT + 1):
                    if prev is not None:
                        kq0, vq0, pk0, t0 = prev
                        for hh in range(2):
                            kv_ps = kvpsp.tile([D, 512], F32)
                            for h8 in range(8):
                                h = hh * 8 + h8
                                nc.tensor.matmul(
                                    kv_ps[:, h8 * D : (h8 + 1) * D],
                                    lhsT=kq0[:, h * D : (h + 1) * D],
                                    rhs=vq0[:, h * D : (h + 1) * D],
                                    start=True,
                                    stop=True,
                                )
                            dst = kvs[0:D, hh * 512 : (hh + 1) * 512]
                            if t0 == 0:
                                nc.vector.tensor_copy(dst, kv_ps[:])
                            else:
                                nc.vector.tensor_add(dst, dst, kv_ps[:])
                        for nh in range(2):
                            nc.tensor.matmul(
                                pk0[0:1, nh * 512 : (nh + 1) * 512],
                                lhsT=onecol[:],
                                rhs=kq0[:, nh * 512 : (nh + 1) * 512],
                                start=True,
                                stop=True,
                            )
                        if t0 == 0:
                            nc.vector.tensor_copy(kvs[D : D + 1, :], pk0[0:1, :])
                        else:
                            nc.vector.tensor_add(
                                kvs[D : D + 1, :], kvs[D : D + 1, :], pk0[0:1, :]
                            )
                    if tt == NT:
                        break
                    if tt == 2:
                        nc.sync.dma_start(
                            wq_sb[:], wqt.ap().rearrange("(c p) n -> p c n", p=128)
                        )
                    if tt == 4:
                        xt2_first = xp2a.tile([128, 8, 512], F32R)
                        nc.sync.dma_start(
                            xt2_first[:],
                            xt.ap().rearrange("(c p) t -> p c t", p=128)[:, :, 0:512],
                        )
                    if tt in pre_x:
                        xtile = pre_x.pop(tt)
                    else:
                        xtile = xp1.tile([128, 8, 128], F32R)
                        nc.sync.dma_start(
                            xtile[:],
                            xt.ap().rearrange("(c p) t -> p c t", p=128)[
                                :, :, tt * 128 : (tt + 1) * 128
                            ],
                        )
                    kq = kvp1.tile([128, C], F32R)
                    vq = kvp1.tile([128, C], F32R)
                    pk_k = None
                    for half in range(2):  # 0 = k, 1 = v
                        pk = ps1.tile([128, C], F32)
                        for nh in range(2):
                            sl = slice(half * C + nh * 512, half * C + (nh + 1) * 512)
                            for ic in range(8):
                                nc.tensor.matmul(
                                    pk[:, nh * 512 : (nh + 1) * 512],
                                    lhsT=xtile[:, ic, :],
                                    rhs=wkv_sb[:, ic, sl],
                                    start=(ic == 0),
                                    stop=(ic == 7 and not with_bias),
                                )
                            if with_bias:
                                nc.tensor.matmul(
                                    pk[:, nh * 512 : (nh + 1) * 512],
                                    lhsT=ones[0:1, 0:128],
                                    rhs=bkv_sb[0:1, sl],
                                    start=False,
                                    stop=True,
                                )
                        if half == 0:
                            pk_k = pk
                            # elu(x)+1 = relu(x) + exp(min(x, 0))
                            krelu = tp1.tile([128, C], F32)
                            nc.scalar.activation(
                                krelu[:], pk[:], mybir.ActivationFunctionType.Relu
                            )
                            kmin = tp1.tile([128, C], F32)
                            nc.vector.tensor_scalar_min(kmin[:], pk[:], 0.0)
                            nc.scalar.activation(
                                kmin[:], kmin[:], mybir.ActivationFunctionType.Exp
                            )
                            nc.vector.tensor_add(kq[:], krelu[:], kmin[:])
                        else:
                            nc.vector.tensor_copy(vq[:], pk[:])
                    prev = (kq, vq, pk_k, tt)

                nc.gpsimd.dma_start(cc_in.ap(), kvs[:])
                nc.gpsimd.collective_compute(
                    "AllReduce",
                    mybir.AluOpType.add,
                    replica_groups=groups,
                    ins=[cc_in.ap().opt()],
                    outs=[cc_out.ap().opt()],
                )
                nc.gpsimd.dma_start(kvt[:], cc_out.ap())
                nc.vector.tensor_copy(ksum_r[:], kvt[D : D + 1, :])
                krp = krpp.tile([64, 512], F32)
                for h in range(H):
                    po = (h % 2) * 64
                    nc.vector.tensor_copy(
                        kvkr[po : po + 64, h * 128 + 64 : (h + 1) * 128],
                        kvt[0:D, h * D : (h + 1) * D],
                    )
                    # krep[d, e] = k_sum_h[d] (broadcast via K=1 matmul)
                    nc.tensor.matmul(
                        krp[:, (h % 8) * D : (h % 8 + 1) * D],
                        lhsT=ksum_r[0:1, h * D : (h + 1) * D],
                        rhs=ones_row[0:1, :],
                        start=True,
                        stop=True,
                    )
                    nc.vector.tensor_copy(
                        kvkr[po : po + 64, h * 128 : h * 128 + 64],
                        krp[:, (h % 8) * D : (h % 8 + 1) * D],
                    )

            # ---------------- stage 2: q projection, y = q'kv / (q'.k_sum), c_proj
            with (
                tc.tile_pool(name="wc", bufs=1) as wcp,
                tc.tile_pool(name="xt2", bufs=1) as xp2,
                tc.tile_pool(name="qc", bufs=3) as qcp,
                tc.tile_pool(name="ytz", bufs=1) as ytzp,
                tc.tile_pool(name="tmp2", bufs=1) as tp2,
                tc.tile_pool(name="zr", bufs=1) as zrp,
                tc.tile_pool(name="osb", bufs=2) as osbp,
                tc.tile_pool(name="psq", bufs=2, space="PSUM") as psq,
                tc.tile_pool(name="psy", bufs=2, space="PSUM") as psy,
                tc.tile_pool(name="pso", bufs=1, space="PSUM") as pso,
            ):
                wc_sb = wcp.tile([128, 8, C], F32R)
                nc.sync.dma_start(
                    wc_sb[:], wct.ap().rearrange("(c p) n -> p c n", p=128)
                )

                prev = None
                for st in range(NST + 1):
                    # y/den matmuls for the previous super-tile first: their
                    # DVE tail (reciprocal + scale) overlaps this super-tile's
                    # q-projection matmuls; c_proj comes after that.
                    ytz = None
                    if prev is not None:
                        qc0, st0 = prev
                        ytz = ytzp.tile([128, 8, 512], F32R)
                        for j in range(8):  # head pairs (2j, 2j+1)
                            yd = psy.tile([128, 2, 512], F32)
                            for jj in range(2):
                                h = 2 * j + jj
                                nc.tensor.matmul(
                                    yd[:, jj, :],
                                    lhsT=kvkr[
                                        jj * 64 : jj * 64 + 64,
                                        h * 128 : (h + 1) * 128,
                                    ],
                                    rhs=qc0[jj * 64 : jj * 64 + 64, j, :],
                                    start=True,
                                    stop=True,
                                )
                            # rows 0:64 = denominators, rows 64:128 = y^T
                            zr = zrp.tile([64, 2, 512], F32)
                            nc.vector.reciprocal_approx_fast(zr[:], yd[0:64, :, :])
                            nc.vector.tensor_mul(
                                ytz[0:64, j, :], yd[64:128, 0, :], zr[:, 0, :]
                            )
                            nc.vector.tensor_mul(
                                ytz[64:128, j, :], yd[64:128, 1, :], zr[:, 1, :]
                            )
                    if st < NST:
                        if st == 0:
                            xtile = xt2_first
                        else:
                            xtile = (xp2a if st % 2 == 0 else xp2).tile(
                                [128, 8, 512], F32R
                            )
                            nc.sync.dma_start(
                                xtile[:],
                                xt.ap().rearrange("(c p) t -> p c t", p=128)[
                                    :, :, st * 512 : (st + 1) * 512
                                ],
                            )
                        qc = qcp.tile([128, 8, 512], F32R)
                        for op in range(4):  # oc pairs
                            qsb = tp2.tile([128, 2, 512], F32)
                            for o2 in range(2):
                                oc = op * 2 + o2
                                qp = psq.tile([128, 512], F32)
                                for ic in range(8):
                                    nc.tensor.matmul(
                                        qp[:],
                                        lhsT=wq_sb[:, ic, oc * 128 : (oc + 1) * 128],
                                        rhs=xtile[:, ic, :],
                                        start=(ic == 0),
                                        stop=(ic == 7 and not with_bias),
                                    )
                                if with_bias:
                                    nc.tensor.matmul(
                                        qp[:],
                                        lhsT=bq_sb[0:1, oc * 128 : (oc + 1) * 128],
                                        rhs=ones[0:1, 0:512],
                                        start=False,
                                        stop=True,
                                    )
                                nc.scalar.copy(qsb[:, o2, :], qp[:])
                            # elu(x)+1 = relu(x) + exp(min(x, 0)) on [128, 1024]
                            qcs = qc[:, op * 2 : op * 2 + 2, :]
                            nc.vector.tensor_scalar_min(qcs, qsb[:], 0.0)
                            nc.scalar.activation(
                                qcs, qcs, mybir.ActivationFunctionType.Exp
                            )
                            nc.vector.tensor_scalar_max(qsb[:], qsb[:], 0.0)
                            nc.vector.tensor_add(qcs, qsb[:], qcs)
                    if ytz is not None:
                        for k in range(4):
                            gt = st0 * 4 + k
                            for ch in range(2):
                                op2 = pso.tile([128, 512], F32)
                                for oc2 in range(8):
                                    nc.tensor.matmul(
                                        op2[:],
                                        lhsT=ytz[:, oc2, k * 128 : (k + 1) * 128],
                                        rhs=wc_sb[:, oc2, ch * 512 : (ch + 1) * 512],
                                        start=(oc2 == 0),
                                        stop=(oc2 == 7 and not with_bias),
                                    )
                                if with_bias:
                                    nc.tensor.matmul(
                                        op2[:],
                                        lhsT=ones[0:1, 0:128],
                                        rhs=bc_sb[0:1, ch * 512 : (ch + 1) * 512],
                                        start=False,
                                        stop=True,
                                    )
                                osb = osbp.tile([128, 512], F32)
                                nc.scalar.copy(osb[:], op2[:])
                                nc.sync.dma_start(
                                    out.ap()[
                                        gt * 128 : (gt + 1) * 128,
                                        ch * 512 : (ch + 1) * 512,
                                    ],
                                    osb[:],
                                )
                    prev = (qc, st) if st < NST else None

    nc.compile()
    _CACHE[key] = nc
    return nc


LAST_RESULT = None


def kernel(x, Wq, bq, Wk, bk, Wv, bv, Wc, bc):
    global LAST_RESULT
    _ensure_hook_shim()
    from concourse.bass_utils import run_bass_kernel_spmd

    bq = np.asarray(bq, np.float32)
    bk = np.asarray(bk, np.float32)
    bv = np.asarray(bv, np.float32)
    bc = np.asarray(bc, np.float32)
    with_bias = bool(
        bq.any() or bk.any() or bv.any() or bc.any()
    )
    nc = _build(with_bias)

    x = np.ascontiguousarray(np.asarray(x, dtype=np.float32))
    xt_full = np.ascontiguousarray(x.reshape(ROWS, C).T)  # [C, ROWS]
    wkvt = np.ascontiguousarray(
        np.concatenate(
            [np.asarray(Wk, np.float32).T, np.asarray(Wv, np.float32).T], axis=1
        )
    )
    wqt = np.ascontiguousarray(np.asarray(Wq, np.float32).T)
    wct = np.ascontiguousarray(np.asarray(Wc, np.float32).T)

    in_maps = []
    for c in range(N_CORES):
        m = {
            "xt": np.ascontiguousarray(xt_full[:, c * RPC : (c + 1) * RPC]),
            "wkvt": wkvt,
            "wqt": wqt,
            "wct": wct,
        }
        if with_bias:
            m["bkv"] = np.concatenate([bk, bv]).reshape(1, 2 * C)
            m["bq"] = bq.reshape(1, C)
            m["bc"] = bc.reshape(1, C)
        in_maps.append(m)

    res = run_bass_kernel_spmd(nc, in_maps, core_ids=list(range(N_CORES)))
    LAST_RESULT = res
    out = np.concatenate([res.results[c]["out"] for c in range(N_CORES)], axis=0)
    return out.reshape(B, T, C)



# revision 8
# speedup vs baseline: 1.5232x; 1.5232x over previous
"""BioLinearAttention (ELU+1 linear attention) on 8 TRN2 NeuronCores.

Sharding: token-parallel. The (B, T) = (4, 4096) grid flattens to 16384 rows;
each core owns 2048 contiguous rows (core c holds batch c//2's half). Each core
computes k/v projections for its rows, accumulates partial kv = k'^T v and
k_sum directly in PSUM across all 16 token tiles, then a pairwise AllReduce
(cores 2b, 2b+1 share batch b) completes the per-batch kv / k_sum. The q
projection for all rows runs while the collective is in flight. Stage C
computes den = q'.k_sum via block-diagonal 2-head matmuls, pre-scales
q~ = q' / den on the DVE, then y^T = kv^T_blockdiag @ q~ (K=128 2-head packed)
and the output projection.

All matmul operands are bf16 (inputs cast host-side); PSUM accumulation stays
fp32. The output is written bf16 and cast back to fp32 on host. DVE element
wise ops run on bf16 SBUF tiles to hit the 2x/4x DVE fast paths; reciprocal
stays fp32 (reciprocal_approx_fast requires it).
"""

import sys
import types

import numpy as np

B, T, C = 4, 4096, 1024
H, D = 16, 64
N_CORES = 8
ROWS = B * T
RPC = ROWS // N_CORES  # rows per core: 2048
NT = RPC // 128  # 128-token tiles per core: 16
NST = RPC // 512  # 512-token super-tiles per core: 4

_CACHE = {}


def _ensure_hook_shim():
    """bass_utils imports antenv.axon_hooks when BASS_TRACE is set; the image
    lacks that module. Provide a no-op shim unless one is already installed."""
    if "antenv.axon_hooks" in sys.modules:
        return
    try:
        import antenv
    except ImportError:
        return
    mod = types.ModuleType("antenv.axon_hooks")
    mod._hook = None
    mod.set_axon_ntff_profile_hook = lambda h: setattr(mod, "_hook", h)
    mod.get_axon_ntff_profile_hook = lambda: mod._hook
    sys.modules["antenv.axon_hooks"] = mod
    antenv.axon_hooks = mod


def _build(with_bias):
    key = ("nc", with_bias)
    if key in _CACHE:
        return _CACHE[key]

    import concourse.bacc as bacc
    import concourse.mybir as mybir
    from concourse.tile import TileContext

    F32 = mybir.dt.float32
    BF16 = mybir.dt.bfloat16
    AF = mybir.ActivationFunctionType

    nc = bacc.Bacc("TRN2", num_devices=N_CORES, debug=False)

    xt = nc.dram_tensor("xt", [C, RPC], BF16, kind="ExternalInput")
    wkvt = nc.dram_tensor("wkvt", [C, 2 * C], BF16, kind="ExternalInput")
    wqt = nc.dram_tensor("wqt", [C, C], BF16, kind="ExternalInput")
    wct = nc.dram_tensor("wct", [C, C], BF16, kind="ExternalInput")
    if with_bias:
        bkv = nc.dram_tensor("bkv", [1, 2 * C], BF16, kind="ExternalInput")
        bq = nc.dram_tensor("bq", [1, C], BF16, kind="ExternalInput")
        bc = nc.dram_tensor("bc", [1, C], BF16, kind="ExternalInput")
    out = nc.dram_tensor("out", [RPC, C], BF16, kind="ExternalOutput")
    # kv (rows 0:64) | k_sum (row 64) partials, layout [d, (h, e)]
    cc_in = nc.dram_tensor("cc_in", [D + 1, C], F32, kind="Internal")
    cc_out = nc.dram_tensor("cc_out", [D + 1, C], F32, kind="Internal")
    groups = [[0, 1], [2, 3], [4, 5], [6, 7]]

    with TileContext(nc) as tc:
        with (
            tc.tile_pool(name="const", bufs=1) as cst,
            tc.tile_pool(name="wts", bufs=1) as wtp,
            tc.tile_pool(name="xres", bufs=1) as xrp,
            tc.tile_pool(name="qres", bufs=1) as qrp,
            tc.tile_pool(name="kvres", bufs=1) as kvp,
        ):
            # ---- persistent SBUF ----
            x_sb = xrp.tile([128, 8, RPC], BF16)  # x^T, c-chunk major
            wkv_sb = wtp.tile([128, 8, 2 * C], BF16)
            wq_sb = wtp.tile([128, 8, C], BF16)
            wc_sb = wtp.tile([128, 8, C], BF16)
            qn_sb = qrp.tile([128, 8, RPC], BF16)  # q' (ELU+1), later q~ in place
            kv2 = kvp.tile([128, 8, 128], BF16)  # block-diag per head pair
            krep2 = kvp.tile([128, 8, 128], BF16)  # block-diag ksum-replicated
            kvt_sb = kvp.tile([D + 1, 8, 128], F32)  # collective result
            ksum_bf = kvp.tile([1, 8, 128], BF16)
            onecol = cst.tile([128, 1], BF16)
            ones_row = cst.tile([1, 512], BF16)
            if with_bias:
                bkv_sb = cst.tile([1, 2 * C], BF16)
                bq_sb = cst.tile([1, C], BF16)
                bc_sb = cst.tile([1, C], BF16)
                nc.sync.dma_start(bkv_sb[:], bkv.ap())
                nc.sync.dma_start(bq_sb[:], bq.ap())
                nc.sync.dma_start(bc_sb[:], bc.ap())

            x_re = xt.ap().rearrange("(c p) t -> p c t", p=128)

            # first two x tiles, then k-weights before the x bulk
            nc.sync.dma_start(x_sb[:, :, 0:128], x_re[:, :, 0:128])
            for nh in range(2):  # k weight halves first: tile 0's k-proj deps
                nc.sync.dma_start(
                    wkv_sb[:, :, nh * 512 : (nh + 1) * 512],
                    wkvt.ap().rearrange("(c p) n -> p c n", p=128)[
                        :, :, nh * 512 : (nh + 1) * 512
                    ],
                )
            nc.sync.dma_start(x_sb[:, :, 128:256], x_re[:, :, 128:256])
            for nh in range(2, 4):
                nc.sync.dma_start(
                    wkv_sb[:, :, nh * 512 : (nh + 1) * 512],
                    wkvt.ap().rearrange("(c p) n -> p c n", p=128)[
                        :, :, nh * 512 : (nh + 1) * 512
                    ],
                )
            nc.sync.dma_start(x_sb[:, :, 256:512], x_re[:, :, 256:512])
            nc.vector.memset(onecol[:], 1.0)
            nc.vector.memset(ones_row[:], 1.0)
            nc.vector.memset(kv2[:], 0.0)
            nc.vector.memset(krep2[:], 0.0)
            for half in range(2):  # rest of x
                nc.sync.dma_start(
                    x_sb[:, :, 512 + half * 768 : 512 + (half + 1) * 768],
                    x_re[:, :, 512 + half * 768 : 512 + (half + 1) * 768],
                )
            nc.sync.dma_start(
                wq_sb[:], wqt.ap().rearrange("(c p) n -> p c n", p=128)
            )
            nc.sync.dma_start(
                wc_sb[:], wct.ap().rearrange("(c p) n -> p c n", p=128)
            )

            # ---------------- stage A: k/v proj, kv & k_sum PSUM accumulation
            with (
                tc.tile_pool(name="kv1", bufs=2) as kvp1,
                tc.tile_pool(name="el1", bufs=2) as el1,
                tc.tile_pool(name="ps1", bufs=6, space="PSUM") as ps1,
                tc.tile_pool(name="kvacc", bufs=1, space="PSUM") as kvap,
            ):
                kvacc = kvap.tile([D + 1, C], F32)

                prev = None
                for tt in range(NT + 1):
                    if prev is not None:
                        kq0, vq0, t0 = prev
                        st_acc = t0 == 0
                        sp_acc = t0 == NT - 1
                        for h in range(H):
                            # start=True marks the whole 2KB PSUM bank row
                            # pending-zero, so only the first head touching
                            # each bank may set it; later heads' first-tile
                            # writes land on pending bytes and init cleanly.
                            nc.tensor.matmul(
                                kvacc[0:D, h * D : (h + 1) * D],
                                lhsT=kq0[:, h * D : (h + 1) * D],
                                rhs=vq0[:, h * D : (h + 1) * D],
                                start=st_acc and h % 8 == 0,
                                stop=sp_acc,
                                skip_group_check=True,
                            )
                        for nh in range(2):
                            nc.tensor.matmul(
                                kvacc[D : D + 1, nh * 512 : (nh + 1) * 512],
                                lhsT=onecol[:],
                                rhs=kq0[:, nh * 512 : (nh + 1) * 512],
                                start=st_acc,
                                stop=sp_acc,
                                skip_group_check=True,
                            )
                    if tt == NT:
                        break
                    xtile = x_sb[:, :, tt * 128 : (tt + 1) * 128]
                    kq = kvp1.tile([128, C], BF16)
                    vq = kvp1.tile([128, C], BF16)
                    for half in range(2):  # 0 = k, 1 = v
                        for nh in range(2):
                            sl = slice(half * C + nh * 512, half * C + (nh + 1) * 512)
                            pk = ps1.tile([128, 512], F32)
                            for ic in range(8):
                                nc.tensor.matmul(
                                    pk[:],
                                    lhsT=xtile[:, ic, :],
                                    rhs=wkv_sb[:, ic, sl],
                                    start=(ic == 0),
                                    stop=(ic == 7 and not with_bias),
                                )
                            if with_bias:
                                nc.tensor.matmul(
                                    pk[:],
                                    lhsT=ones_row[0:1, 0:128],
                                    rhs=bkv_sb[0:1, sl],
                                    start=False,
                                    stop=True,
                                )
                            osl = slice(nh * 512, (nh + 1) * 512)
                            if half == 0:
                                # elu(x)+1 = relu(x) + exp(min(x, 0))
                                krelu = el1.tile([128, 512], BF16, tag="krelu")
                                nc.scalar.activation(krelu[:], pk[:], AF.Relu)
                                kmin = el1.tile([128, 512], BF16, tag="kmin")
                                nc.vector.tensor_scalar_min(kmin[:], pk[:], 0.0)
                                nc.scalar.activation(kmin[:], kmin[:], AF.Exp)
                                nc.vector.tensor_add(kq[:, osl], krelu[:], kmin[:])
                            else:
                                nc.scalar.copy(vq[:, osl], pk[:])
                    prev = (kq, vq, tt)

                # kickoff AllReduce of [kv | k_sum]
                kvs = el1.tile([D + 1, C], F32, tag="kvs", bufs=1)
                nc.scalar.copy(kvs[:], kvacc[:])
                nc.gpsimd.dma_start(cc_in.ap(), kvs[:])
                nc.gpsimd.collective_compute(
                    "AllReduce",
                    mybir.AluOpType.add,
                    replica_groups=groups,
                    ins=[cc_in.ap().opt()],
                    outs=[cc_out.ap().opt()],
                )
                nc.gpsimd.dma_start(
                    kvt_sb[:], cc_out.ap().rearrange("p (c n) -> p c n", c=8)
                )

            # ---------------- stage B: q projection (overlaps the collective)
            with (
                tc.tile_pool(name="el2", bufs=3) as el2,
                tc.tile_pool(name="zpool", bufs=3) as zp,
                tc.tile_pool(name="ytz", bufs=2) as ytzp,
                tc.tile_pool(name="osb", bufs=3) as osbp,
                tc.tile_pool(name="psq", bufs=2, space="PSUM") as psq,
                tc.tile_pool(name="psden", bufs=2, space="PSUM") as psden,
                tc.tile_pool(name="psy", bufs=2, space="PSUM") as psy,
                tc.tile_pool(name="pso", bufs=2, space="PSUM") as pso,
            ):
                for st in range(NST):
                    tsl = slice(st * 512, (st + 1) * 512)
                    for oc in range(8):
                        qp = psq.tile([128, 512], F32)
                        for ic in range(8):
                            nc.tensor.matmul(
                                qp[:],
                                lhsT=wq_sb[:, ic, oc * 128 : (oc + 1) * 128],
                                rhs=x_sb[:, ic, tsl],
                                start=(ic == 0),
                                stop=(ic == 7 and not with_bias),
                            )
                        if with_bias:
                            nc.tensor.matmul(
                                qp[:],
                                lhsT=bq_sb[0:1, oc * 128 : (oc + 1) * 128],
                                rhs=ones_row[0:1, :],
                                start=False,
                                stop=True,
                            )
                        # elu(x)+1 = relu(x) + exp(min(x, 0))
                        qrelu = el2.tile([128, 512], BF16, tag="qrelu")
                        nc.scalar.activation(qrelu[:], qp[:], AF.Relu)
                        qmin = el2.tile([128, 512], BF16, tag="qmin")
                        nc.vector.tensor_scalar_min(qmin[:], qp[:], 0.0)
                        nc.scalar.activation(qmin[:], qmin[:], AF.Exp)
                        nc.vector.tensor_add(
                            qn_sb[:, oc, tsl], qrelu[:], qmin[:]
                        )

                # ---- kv2 / krep2 assembly (waits on the collective) ----
                nc.scalar.copy(ksum_bf[:], kvt_sb[D : D + 1, :, :])
                krp = psden.tile([128, 8, 64], F32, tag="dps")
                for h in range(H):
                    po = (h % 2) * 64
                    # one bank: only the first write per partition half may
                    # set start (bank-granular pending-zero, as above)
                    nc.tensor.matmul(
                        krp[po : po + 64, h // 2, :],
                        lhsT=ksum_bf[0:1, h // 2, po : po + 64],
                        rhs=ones_row[0:1, 0:64],
                        start=h < 2,
                        stop=True,
                        skip_group_check=True,
                    )
                # scatter kv and krep into zero-padded block-diagonal pair form
                for po in range(2):  # even heads -> rows 0:64, odd -> 64:128
                    sl64 = slice(po * 64, po * 64 + 64)
                    nc.vector.tensor_copy(
                        kv2[sl64, :, sl64],
                        kvt_sb[0:D, :, po * 64 : po * 64 + 64],
                    )
                    nc.vector.tensor_copy(
                        krep2[sl64, :, sl64],
                        krp[sl64, :, :],
                    )

                # den for all supertiles; recip + in-place q~ = q' * z
                for st in range(NST):
                    tsl = slice(st * 512, (st + 1) * 512)
                    for j in range(8):
                        dps = psden.tile([128, 512], F32)
                        nc.tensor.matmul(
                            dps[:],
                            lhsT=krep2[:, j, :],
                            rhs=qn_sb[:, j, tsl],
                            start=True,
                            stop=True,
                        )
                        z32 = zp.tile([128, 512], F32, tag="z32")
                        nc.vector.reciprocal_approx_fast(z32[:], dps[:])
                        z16 = zp.tile([128, 512], BF16, tag="z16")
                        nc.scalar.copy(z16[:], z32[:])
                        nc.vector.tensor_mul(
                            qn_sb[:, j, tsl], qn_sb[:, j, tsl], z16[:]
                        )

                # ---- stage C: y^T = blockdiag(kv)^T q~, then c_proj ----
                for st in range(NST):
                    tsl = slice(st * 512, (st + 1) * 512)
                    ytz = ytzp.tile([128, 8, 512], BF16)
                    for j in range(8):
                        yps = psy.tile([128, 512], F32)
                        nc.tensor.matmul(
                            yps[:],
                            lhsT=kv2[:, j, :],
                            rhs=qn_sb[:, j, tsl],
                            start=True,
                            stop=True,
                        )
                        nc.scalar.copy(ytz[:, j, :], yps[:])
                    for k in range(4):
                        gt = st * 4 + k
                        for ch in range(2):
                            op2 = pso.tile([128, 512], F32)
                            for oc2 in range(8):
                                nc.tensor.matmul(
                                    op2[:],
                                    lhsT=ytz[:, oc2, k * 128 : (k + 1) * 128],
                                    rhs=wc_sb[:, oc2, ch * 512 : (ch + 1) * 512],
                                    start=(oc2 == 0),
                                    stop=(oc2 == 7 and not with_bias),
                                )
                            if with_bias:
                                nc.tensor.matmul(
                                    op2[:],
                                    lhsT=ones_row[0:1, 0:128],
                                    rhs=bc_sb[0:1, ch * 512 : (ch + 1) * 512],
                                    start=False,
                                    stop=True,
                                )
                            osb = osbp.tile([128, 512], BF16)
                            nc.scalar.copy(osb[:], op2[:])
                            nc.sync.dma_start(
                                out.ap()[
                                    gt * 128 : (gt + 1) * 128,
                                    ch * 512 : (ch + 1) * 512,
                                ],
                                osb[:],
                            )

    nc.compile()
    _CACHE[key] = nc
    return nc


LAST_RESULT = None


def kernel(x, Wq, bq, Wk, bk, Wv, bv, Wc, bc):
    global LAST_RESULT
    _ensure_hook_shim()
    import ml_dtypes
    from concourse.bass_utils import run_bass_kernel_spmd

    BF = ml_dtypes.bfloat16

    bq = np.asarray(bq, np.float32)
    bk = np.asarray(bk, np.float32)
    bv = np.asarray(bv, np.float32)
    bc = np.asarray(bc, np.float32)
    with_bias = bool(bq.any() or bk.any() or bv.any() or bc.any())
    nc = _build(with_bias)

    x = np.ascontiguousarray(np.asarray(x, dtype=np.float32))
    xt_full = np.ascontiguousarray(x.reshape(ROWS, C).T.astype(BF))  # [C, ROWS]
    wkvt = np.ascontiguousarray(
        np.concatenate(
            [np.asarray(Wk, np.float32).T, np.asarray(Wv, np.float32).T], axis=1
        ).astype(BF)
    )
    wqt = np.ascontiguousarray(np.asarray(Wq, np.float32).T.astype(BF))
    wct = np.ascontiguousarray(np.asarray(Wc, np.float32).T.astype(BF))

    in_maps = []
    for c in range(N_CORES):
        m = {
            "xt": np.ascontiguousarray(xt_full[:, c * RPC : (c + 1) * RPC]),
            "wkvt": wkvt,
            "wqt": wqt,
            "wct": wct,
        }
        if with_bias:
            m["bkv"] = np.concatenate([bk, bv]).reshape(1, 2 * C).astype(BF)
            m["bq"] = bq.reshape(1, C).astype(BF)
            m["bc"] = bc.reshape(1, C).astype(BF)
        in_maps.append(m)

    res = run_bass_kernel_spmd(nc, in_maps, core_ids=list(range(N_CORES)))
    LAST_RESULT = res
    out = np.concatenate(
        [
            np.asarray(res.results[c]["out"]).astype(np.float32)
            for c in range(N_CORES)
        ],
        axis=0,
    )
    return out.reshape(B, T, C)
